# revision 5
# baseline (speedup 1.0000x reference)
"""Bidirectional 2-layer LSTM (B=256, T=128, EMB=256, HS=512, VS=64) on 8 trn2 cores.

v2: loop-structured, minimal-instruction-count design.

Sharding: 4-way data-parallel over batch x 2-way direction split (core = 2q+d;
bwd cores get time-reversed one-hot input and the W_b* weights, identical SPMD
program). Per core: batch 64, both LSTM layers.

Device program per core:
  - L0 and L1 cells stacked on PSUM partition halves (L0 batch rows 0:64,
    L1 rows 64:128 via tile_position), gates [128, 2048] fp32 in 4 PSUM banks.
    Gate column order host-permuted to [i f o g].
  - One fused step: 52 matmuls (N=512, K<=128), 4 activations/3 tensor ops for
    BOTH cells (fused across the partition stack), 4 DMA transposes of the
    stacked h -> hT ring, 2 small DMAs.
  - L1 runs one step behind L0 (classic interleave) so all stationary operands
    come from one hT ring tile; the whole T-loop is a single hardware For_i
    (body = 2 unrolled steps for static ring ping-pong; only the one-hot load
    and the h1T sequence store use loop-index APs).
  - h1T for all t accumulates in SBUF; epilogue computes the transposed
    compress partial P^T = Wc_d^T h1T in one hardware loop, exchanges partials
    with the pair core in ONE AllGather, then combines (add + tanh + fc ->
    transposed logits) unrolled with a negative-stride AP doing the bwd time
    reversal.
"""

import os
import sys
from contextlib import ExitStack

import numpy as np
import ml_dtypes

for _p in ("/opt/trn_rl_repo",):
    if _p not in sys.path and os.path.isdir(_p):
        sys.path.insert(0, _p)

os.environ.setdefault("JAX_COMPILATION_CACHE_DIR", "/tmp/jaxcache")
os.environ.setdefault("JAX_PERSISTENT_CACHE_MIN_COMPILE_TIME_SECS", "1")

B, T, VS, EMB, HS = 256, 128, 64, 256, 512
NCORES = 8
BC = 64          # batch per core
G4 = 4 * HS      # 2048 gate dims
CHUNK = 8        # timesteps per compress chunk

BF16 = ml_dtypes.bfloat16

_PAIRS = [[0, 1], [2, 3], [4, 5], [6, 7]]

# gate row order [i f o g] (reference raw order is [i f g o])
_PERM = np.concatenate([
    np.arange(0, 512),        # i
    np.arange(512, 1024),     # f
    np.arange(1536, 2048),    # o
    np.arange(1024, 1536),    # g
])


def build_program(with_gb1, t_steps=T, mode="loop"):
    import concourse.bass as bass  # noqa: F401
    import concourse.mybir as mybir
    import concourse.tile as tile
    from concourse import bacc

    f32 = mybir.dt.float32
    bf16 = mybir.dt.bfloat16
    AF = mybir.ActivationFunctionType
    Tn = t_steps
    assert Tn % CHUNK == 0 and Tn % 2 == 0
    NCH = Tn // CHUNK
    NS = Tn + 2  # one-hot slots (slot Tn, Tn+1 are zero padding)

    nc = bacc.Bacc()

    # ---- I/O ----
    ohT = nc.dram_tensor("ohT", [64, NS * 64], bf16, kind="ExternalInput")
    g0tab = nc.dram_tensor("g0tab", [64, G4], bf16, kind="ExternalInput")
    wh0 = nc.dram_tensor("wh0", [4, 128, G4], bf16, kind="ExternalInput")
    wx1 = nc.dram_tensor("wx1", [4, 128, G4], bf16, kind="ExternalInput")
    wh1 = nc.dram_tensor("wh1", [4, 128, G4], bf16, kind="ExternalInput")
    wcT = nc.dram_tensor("wcT", [4, 128, 512], bf16, kind="ExternalInput")
    fct = nc.dram_tensor("fct", [4, 128, 64], bf16, kind="ExternalInput")
    cbias = nc.dram_tensor("cbias", [128, 4], f32, kind="ExternalInput")
    fbias = nc.dram_tensor("fbias", [64, 1], f32, kind="ExternalInput")
    if with_gb1:
        gb1 = nc.dram_tensor("gb1", [1, G4], bf16, kind="ExternalInput")
    logT = nc.dram_tensor("logT", [64, Tn * 64], f32, kind="ExternalOutput")

    pt_self = nc.dram_tensor("pt_self", [NCH, 128, 2048], bf16)
    pt_both = nc.dram_tensor("pt_both", [2, NCH, 128, 2048], bf16)

    if os.environ.get("BLSTM_NULL", "0") == "1":
        # Null calibration program: must CONSUME every input so the
        # host->device transfers are not pruned -- the null should differ
        # from the real program only in the computation being measured.
        with tile.TileContext(nc) as tc, ExitStack() as ctx:
            pool = ctx.enter_context(tc.tile_pool(name="np", bufs=1))
            z = pool.tile([64, 512], f32, name="z")
            nc.vector.memset(z, 0.0)
            touch = pool.tile([128, 64], bf16, name="touch")
            srcs = [ohT[:, 0:3], g0tab[:, 0:3], wcT[0][:, 0:3], fct[0][:, 0:3]]
            srcs += [w[k][:, 0:3] for w in (wh0, wx1, wh1) for k in range(4)]
            if with_gb1:
                srcs.append(gb1[:, 0:3])
            for i, src in enumerate(srcs):
                rows = src.shape[0]
                nc.sync.dma_start(out=touch[0:rows, 3 * i: 3 * i + 3], in_=src)
            touch2 = pool.tile([128, 8], f32, name="touch2")
            nc.sync.dma_start(out=touch2[:, 0:4], in_=cbias[:, :])
            nc.sync.dma_start(out=touch2[0:64, 4:5], in_=fbias[:, :])
            nc.vector.tensor_copy(z[0:64, 0:60], touch[0:64, 0:60])
            nc.vector.tensor_copy(z[0:64, 60:68], touch2[0:64, :])
            nc.sync.dma_start(out=logT[:, 0:512], in_=z)
        nc.finalize()
        return nc

    with tile.TileContext(nc) as tc, ExitStack() as ctx:
        wpool = ctx.enter_context(tc.tile_pool(name="weights", bufs=1))
        spool = ctx.enter_context(tc.tile_pool(name="state", bufs=1))

        # ---- load weights ----
        ohT_s = wpool.tile([64, NS * 64], bf16, tag="ohT")
        nc.sync.dma_start(out=ohT_s, in_=ohT[:, :])
        g0tab_s = wpool.tile([64, G4], bf16, tag="g0tab")
        nc.sync.dma_start(out=g0tab_s, in_=g0tab[:, :])

        def load(dram, n, cols, tag):
            ts_ = []
            for k in range(n):
                t_ = wpool.tile([128, cols], bf16, tag=f"{tag}{k}", name=f"{tag}{k}")
                nc.sync.dma_start(out=t_, in_=dram[k])
                ts_.append(t_)
            return ts_

        wh0_s = load(wh0, 4, G4, "wh0")
        wx1_s = load(wx1, 4, G4, "wx1")
        wh1_s = load(wh1, 4, G4, "wh1")
        wcT_s = load(wcT, 4, 512, "wcT")
        fct_s = load(fct, 4, 64, "fct")
        cbias_s = wpool.tile([128, 4], f32, tag="cbias")
        nc.sync.dma_start(out=cbias_s, in_=cbias[:, :])
        fbias_s = wpool.tile([64, 1], f32, tag="fbias")
        nc.sync.dma_start(out=fbias_s, in_=fbias[:, :])
        if with_gb1:
            gb1_s = wpool.tile([1, G4], bf16, tag="gb1")
            nc.sync.dma_start(out=gb1_s, in_=gb1[:, :])
            ones_s = wpool.tile([1, 64], bf16, tag="ones")
            nc.vector.memset(ones_s, 1.0)

        # ---- persistent state ----
        # H1A[p, (j, slot, c)]: h1T block j, timestep slot, batch col c.
        H1A = spool.tile([128, 4 * Tn * 64], bf16, tag="H1A")
        ring = [spool.tile([128, 512], bf16, tag=f"ring{i}", name=f"ring{i}") for i in range(2)]
        X = [spool.tile([128, 1024], f32, tag=f"X{i}", name=f"X{i}") for i in range(2)]

        # ---- main loop scope (its PSUM frees before the epilogue) ----
        lctx = ExitStack()
        work = lctx.enter_context(tc.tile_pool(name="work", bufs=1))
        gpool = lctx.enter_context(tc.tile_pool(name="gp", bufs=1, space="PSUM"))
        S = work.tile([128, 1536], bf16, tag="S")
        prod = work.tile([128, 1024], f32, tag="prod")
        TC = work.tile([128, 512], bf16, tag="TC")
        Hh = work.tile([128, 512], bf16, tag="Hh")
        oh_scr = work.tile([64, 64], bf16, tag="ohscr")
        gp = gpool.tile([128, 2048], f32, tag="gp")

        for r in ring:
            nc.vector.memset(r, 0.0)
        for x_ in X:
            nc.vector.memset(x_, 0.0)

        def gates(u_par, l0_only=False):
            """Emit the gate matmuls for one step. ringP = ring[1-u_par] holds
            [h0[u-1] | h1[u-2]] transposed; oh_scr holds the one-hot slice."""
            ringP = ring[1 - u_par]
            # starts first (guard against any bank-clear semantics)
            for n in range(4):
                nc.tensor.matmul(
                    gp[0:64, 512 * n: 512 * n + 512],
                    lhsT=oh_scr,
                    rhs=g0tab_s[:, 512 * n: 512 * n + 512],
                    start=True, stop=False, tile_position=(0, 0),
                )
            if not l0_only:
                if with_gb1:
                    for n in range(4):
                        nc.tensor.matmul(
                            gp[64:128, 512 * n: 512 * n + 512],
                            lhsT=ones_s,
                            rhs=gb1_s[:, 512 * n: 512 * n + 512],
                            start=True, stop=False, tile_position=(0, 64),
                        )
                for j in range(4):
                    for n in range(4):
                        nc.tensor.matmul(
                            gp[64:128, 512 * n: 512 * n + 512],
                            lhsT=ringP[:, 128 * j: 128 * j + 64],
                            rhs=wx1_s[j][:, 512 * n: 512 * n + 512],
                            start=(j == 0 and not with_gb1), stop=False,
                            tile_position=(0, 64),
                        )
            for j in range(4):
                for n in range(4):
                    nc.tensor.matmul(
                        gp[0:64, 512 * n: 512 * n + 512],
                        lhsT=ringP[:, 128 * j: 128 * j + 64],
                        rhs=wh0_s[j][:, 512 * n: 512 * n + 512],
                        start=False, stop=(j == 3), tile_position=(0, 0),
                    )
            if not l0_only:
                for j in range(4):
                    for n in range(4):
                        nc.tensor.matmul(
                            gp[64:128, 512 * n: 512 * n + 512],
                            lhsT=ringP[:, 128 * j + 64: 128 * j + 128],
                            rhs=wh1_s[j][:, 512 * n: 512 * n + 512],
                            start=False, stop=(j == 3), tile_position=(0, 64),
                        )

        def cell_and_transpose(u_par, rows=None):
            """Fused cell for both stacked layers + h transpose into ring."""
            lo, hi = (0, 128) if rows is None else rows
            Xc = X[u_par]
            Xn = X[1 - u_par]
            ringC = ring[u_par]
            nc.scalar.activation(S[lo:hi, :], gp[lo:hi, 0:1536], AF.Sigmoid)
            nc.scalar.activation(Xc[lo:hi, 0:512], gp[lo:hi, 1536:2048], AF.Tanh)
            nc.vector.tensor_mul(prod[lo:hi, :], S[lo:hi, 0:1024], Xc[lo:hi, :])
            nc.vector.tensor_add(
                Xn[lo:hi, 512:1024], prod[lo:hi, 0:512], prod[lo:hi, 512:1024]
            )
            nc.scalar.activation(TC[lo:hi, :], Xn[lo:hi, 512:1024], AF.Tanh)
            nc.vector.tensor_mul(Hh[lo:hi, :], S[lo:hi, 1024:1536], TC[lo:hi, :])
            for j in range(4):
                nc.sync.dma_start_transpose(
                    out=ringC[:, 128 * j + lo: 128 * j + hi],
                    in_=Hh[lo:hi, 128 * j: 128 * j + 128],
                )

        ring_h1 = [
            r.rearrange("p (j half c) -> p j half c", j=4, half=2, c=64)
            for r in ring
        ]

        # ---- peel step u=0 (L0 only; L1 state stays zero) ----
        nc.sync.dma_start(out=oh_scr, in_=ohT_s[:, 0:64])
        gates(0, l0_only=True)
        cell_and_transpose(0, rows=(0, 64))

        # ---- main loop u = 1..Tn ----
        ohTs = ohT_s[:, 64: 64 + Tn * 64].rearrange(
            "v (pr u2 c) -> v pr u2 c", pr=Tn // 2, u2=2, c=64
        )
        H1A5 = H1A.rearrange(
            "p (j pr u2 c) -> p j pr u2 c", j=4, pr=Tn // 2, u2=2, c=64
        )

        def step_body(u_par, oh_src, h1a_dst):
            nc.sync.dma_start(out=oh_scr, in_=oh_src)
            gates(u_par)
            cell_and_transpose(u_par)
            # ring[u_par] now holds [h0[u] | h1[u-1]]; store h1 cols to slot u-1
            nc.sync.dma_start(out=h1a_dst, in_=ring_h1[u_par][:, :, 1, :])

        if mode == "loop":
            with tc.For_i(0, Tn // 2) as i:
                # u = 2i+1 (parity 1), then u = 2i+2 (parity 0)
                step_body(1, ohTs[:, i, 0, :], H1A5[:, :, i, 0, :])
                step_body(0, ohTs[:, i, 1, :], H1A5[:, :, i, 1, :])
        else:
            H1A4 = H1A.rearrange("p (j s c) -> p j s c", j=4, s=Tn, c=64)
            for u in range(1, Tn + 1):
                step_body(
                    u % 2,
                    ohT_s[:, 64 * u: 64 * u + 64],
                    H1A4[:, :, u - 1, :],
                )

        lctx.close()

        # ---- epilogue: compress partials, exchange, combine + fc ----
        H1A6 = H1A.rearrange(
            "p (j cc tt c) -> p j cc tt c", j=4, cc=NCH, tt=CHUNK, c=64
        )
        with tc.tile_pool(name="epi", bufs=1) as epool, \
                tc.tile_pool(name="epips", bufs=1, space="PSUM") as eps:
            gpc = eps.tile([128, 2048], f32, tag="gpc")
            Pc = epool.tile([128, 2048], bf16, tag="Pc")

            def compress_chunk(ci):
                for j in range(4):
                    for pg in range(4):
                        nc.tensor.matmul(
                            gpc[:, 512 * pg: 512 * pg + 512],
                            lhsT=wcT_s[j][:, 128 * pg: 128 * pg + 128],
                            rhs=H1A6[:, j, ci, :, :],
                            start=(j == 0), stop=(j == 3),
                            tile_position=(0, 0),
                        )
                nc.vector.tensor_copy(Pc, gpc)
                nc.sync.dma_start(out=pt_self[ci], in_=Pc)

            if mode == "loop":
                with tc.For_i(0, NCH) as ci:
                    compress_chunk(ci)
            else:
                for ci in range(NCH):
                    compress_chunk(ci)

            import concourse.mybir as mybir2
            nc.gpsimd.collective_compute(
                "AllGather",
                mybir2.AluOpType.bypass,
                replica_groups=_PAIRS,
                ins=[pt_self[:, :, :]],
                outs=[pt_both[:, :, :, :]],
            )

            Pf = epool.tile([128, 2048], bf16, tag="Pf")
            Pb = epool.tile([128, 2048], bf16, tag="Pb")
            Zc = epool.tile([128, 2048], bf16, tag="Zc")
            lg = eps.tile([64, 512], f32, tag="lg")
            lgs = epool.tile([64, 512], f32, tag="lgs")

            for c in range(NCH):
                nc.sync.dma_start(out=Pf, in_=pt_both[0, c])
                rev = pt_both[1, NCH - 1 - c].rearrange(
                    "p (pg tt b) -> p pg tt b", pg=4, tt=CHUNK, b=64
                )[:, :, ::-1, :]
                nc.sync.dma_start(out=Pb, in_=rev)
                nc.vector.tensor_add(Zc, Pf, Pb)
                for pg in range(4):
                    nc.scalar.activation(
                        Zc[:, 512 * pg: 512 * pg + 512],
                        Zc[:, 512 * pg: 512 * pg + 512],
                        AF.Tanh,
                        bias=cbias_s[:, pg: pg + 1],
                    )
                for pg in range(4):
                    nc.tensor.matmul(
                        lg,
                        lhsT=fct_s[pg],
                        rhs=Zc[:, 512 * pg: 512 * pg + 512],
                        start=(pg == 0), stop=(pg == 3),
                        tile_position=(0, 0),
                    )
                nc.scalar.activation(lgs, lg, AF.Identity, bias=fbias_s[:, 0:1])
                nc.sync.dma_start(out=logT[:, 512 * c: 512 * c + 512], in_=lgs)

    nc.finalize()
    return nc


_prog_cache = {}


def _get_program(key):
    if key not in _prog_cache:
        _prog_cache[key] = build_program(*key[:3])
    return _prog_cache[key]


def _prep_core_inputs(x, emb_table, Ws, bs, compress_W, compress_b, fc_W, fc_b,
                      quarter, direction, t_steps=T):
    NS = t_steps + 2
    xq = np.asarray(x[quarter * BC: (quarter + 1) * BC, :t_steps]).astype(np.int64)
    if direction == 1:
        xq = xq[:, ::-1]
    # one-hot^T with slot padding: ohT[v, u*64+b] = (xq[b, u] == v), u < t_steps
    xs = xq.T.reshape(-1)
    ohv = np.zeros((64, NS * 64), dtype=np.float32)
    ohv[xs, np.arange(t_steps * BC)] = 1.0

    W0, W1 = Ws
    b0, b1 = bs
    W0r = np.asarray(W0)[_PERM]               # [2048, EMB+HS]
    W1r = np.asarray(W1)[_PERM]               # [2048, 2*HS]
    g0v = np.asarray(emb_table, dtype=np.float32) @ W0r[:, :EMB].T.astype(np.float32)
    g0v = g0v + np.asarray(b0, dtype=np.float32)[_PERM][None, :]
    wh0v = W0r[:, EMB:].T.reshape(4, 128, G4)
    wx1v = W1r[:, :HS].T.reshape(4, 128, G4)
    wh1v = W1r[:, HS:].T.reshape(4, 128, G4)

    Wc_d = np.asarray(compress_W, dtype=np.float32)[:, direction * HS: (direction + 1) * HS]
    wcTv = Wc_d.T.reshape(4, 128, 512)        # [j][p, out]
    fctv = np.asarray(fc_W, dtype=np.float32).T.reshape(4, 128, 64)
    cbv = np.asarray(compress_b, dtype=np.float32).reshape(4, 128).T.copy()
    fbv = np.asarray(fc_b, dtype=np.float32).reshape(64, 1)

    inmap = {
        "ohT": ohv.astype(BF16),
        "g0tab": g0v.astype(BF16),
        "wh0": wh0v.astype(BF16),
        "wx1": wx1v.astype(BF16),
        "wh1": wh1v.astype(BF16),
        "wcT": wcTv.astype(BF16),
        "fct": fctv.astype(BF16),
        "cbias": cbv,
        "fbias": fbv,
    }
    if np.any(np.asarray(b1)):
        inmap["gb1"] = np.asarray(b1, dtype=np.float32)[_PERM].reshape(1, G4).astype(BF16)
    return inmap


def _run(inputs, trace=False, t_steps=T):
    from concourse.bass_utils import run_bass_kernel_spmd

    x = np.asarray(inputs["x"])
    emb_table = np.asarray(inputs["emb_table"], dtype=np.float32)
    with_gb1 = bool(np.any(np.asarray(inputs["b_f1"])) or np.any(np.asarray(inputs["b_b1"])))
    mode = os.environ.get("BLSTM_MODE", "loop")
    is_null = os.environ.get("BLSTM_NULL", "0") == "1"
    nc = _get_program((with_gb1, t_steps, mode, is_null))

    in_maps = []
    for core in range(NCORES):
        q, d = core // 2, core % 2
        Ws = (inputs["W_f0"], inputs["W_f1"]) if d == 0 else (inputs["W_b0"], inputs["W_b1"])
        bs = (inputs["b_f0"], inputs["b_f1"]) if d == 0 else (inputs["b_b0"], inputs["b_b1"])
        im = _prep_core_inputs(
            x, emb_table, Ws, bs, inputs["compress_W"], inputs["compress_b"],
            inputs["fc_W"], inputs["fc_b"], q, d, t_steps,
        )
        if with_gb1 and "gb1" not in im:
            im["gb1"] = np.zeros((1, G4), dtype=BF16)
        in_maps.append(im)

    res = run_bass_kernel_spmd(nc, in_maps, core_ids=list(range(NCORES)), trace=trace)

    out = np.empty((B, t_steps, VS), dtype=np.float32)
    for q in range(4):
        logT = res.results[2 * q]["logT"]
        out[q * BC: (q + 1) * BC] = logT.reshape(VS, t_steps, BC).transpose(2, 1, 0)
    return out, res


def kernel(**inputs):
    out, _ = _run(inputs, trace=False)
    return out
